# revision 6
# baseline (speedup 1.0000x reference)
"""Trainium2 Bass kernel: disparity regression via top-2 over the last axis.

pred[b, n] = i1 + (i2 - i1) * sigmoid(v2 - v1)  with (v1,i1),(v2,i2) the top-2
of cost[b, n, :192] (ties -> lowest index, matching jax.lax.top_k).

Packed-key single-pass selection: overwrite the low byte of each fp32
element with (255 - d) via a strided uint8 tensor_copy (Pool engine), so
    key = (bits(x) & ~0xFF) | (255 - d)
Keys still compare like the values themselves under fp32 compare (the low
mantissa byte only perturbs the value by <= 256 ulp ~ 3e-5 relative), so a
single DVE max8 pass per row returns the top-2 values AND their indices:
i = 255 - (key & 0xFF), value ~= float(key | 0xFF). Quantization ties
resolve toward the lowest index for positive values (and toward the highest
for negative ones, where the reconstructed weights are ~0.5 anyway).

This needs no separate value pass (max_index), no fp offset pass, and no
int32 bitwise ops off-DVE (Pool only does the byte copy; int32 bitwise is
DVE-only on trn2). Per-core engine budget: DVE ~142us (max8 + tiny
epilogue), Pool ~90us, ACT ~10us, DMA ~152us -> memory-bound.
"""
import numpy as np

import concourse.bacc as bacc
import concourse.tile as tile
import concourse.mybir as mybir
from concourse.bass_utils import run_bass_kernel_spmd

N_CORES = 8
B, N, D = 4, 131072, 192
ROWS = B * N                       # 524288
ROWS_PER_CORE = ROWS // N_CORES    # 65536
P = 128                            # SBUF partitions
G = 16                             # rows per partition per tile
TILE_ROWS = P * G                  # 2048
NQ = 2                             # DMA chunks per tile

F32 = mybir.dt.float32
I32 = mybir.dt.int32
U8 = mybir.dt.uint8
AF = mybir.ActivationFunctionType
OP = mybir.AluOpType


def build(loop_iters: int = 1, rows_per_core: int = ROWS_PER_CORE):
    nc = bacc.Bacc(
        "TRN2", target_bir_lowering=False, debug=False, num_devices=N_CORES
    )
    n_tiles = rows_per_core // TILE_ROWS
    rpp = rows_per_core // P  # rows per partition (contiguous run in DRAM)
    x = nc.dram_tensor("cost", [rows_per_core, D], F32, kind="ExternalInput").ap()
    y = nc.dram_tensor("pred", [rows_per_core], F32, kind="ExternalOutput").ap()

    # partition-major: partition p owns rows [p*rpp, (p+1)*rpp); tile t covers
    # rows p*rpp + t*G .. +G of every partition -> output is one contiguous
    # rpp-element run per partition (single efficient DMA per iteration).
    x_t = x.rearrange("(p t g) d -> t p (g d)", p=P, g=G)
    y_p = y.rearrange("(p r) -> p r", p=P)

    GD = G * D

    def body(tc, pat8, ob):
        pat_b = pat8[:].unsqueeze(1).to_broadcast([P, G, D])
        with (
            tc.tile_pool(name="xp", bufs=3) as xp,
            tc.tile_pool(name="sp", bufs=3) as sp,
            tc.tile_pool(name="ep", bufs=3) as ep,
        ):
            for t in range(n_tiles):
                xt = xp.tile([P, GD], F32)
                for q in range(NQ):
                    c0, c1 = q * GD // NQ, (q + 1) * GD // NQ
                    nc.sync.dma_start(xt[:, c0:c1], x_t[t][:, c0:c1])

                # low byte := 255 - d  (little-endian byte 0 of each fp32)
                xb = xt[:].bitcast(U8).rearrange("p (g d four) -> p g d four",
                                                 d=D, four=4)
                nc.gpsimd.tensor_copy(xb[:, :, :, 0], pat_b)

                # one max8 pass per row: top-2 packed keys in slots 0,1
                k8 = sp.tile([P, G * 8], F32)
                for g in range(G):
                    nc.vector.max(k8[:, g * 8:(g + 1) * 8],
                                  xt[:, g * D:(g + 1) * D])

                k12 = k8[:].rearrange("p (g k) -> p g k", k=8)[:, :, 0:2]
                k12i = k12.bitcast(I32)

                # value part: vq = float(key | 0xFF);  dq = vq2 - vq1 (<= 0)
                mb = ep.tile([P, G * 2], I32)
                mb3 = mb[:].rearrange("p (g k) -> p g k", k=2)
                nc.vector.tensor_scalar(mb3, k12i, 0xFF, None, OP.bitwise_or)
                mbf = mb[:].bitcast(F32).rearrange("p (g k) -> p g k", k=2)
                dq = ep.tile([P, G], F32)
                nc.gpsimd.tensor_sub(dq[:], mbf[:, :, 1], mbf[:, :, 0])
                s = ep.tile([P, G], F32)
                nc.scalar.activation(s[:], dq[:], AF.Sigmoid)

                # index part: u = key & 0xFF;  i = 255 - u
                u = ep.tile([P, G * 2], I32)
                u3 = u[:].rearrange("p (g k) -> p g k", k=2)
                nc.vector.tensor_scalar(u3, k12i, 0xFF, None, OP.bitwise_and)
                uf = ep.tile([P, G * 2], F32)
                uf3 = uf[:].rearrange("p (g k) -> p g k", k=2)
                nc.vector.tensor_copy(uf3, u3)

                # pred = (255-u1) + (u1-u2)*s accumulated into the out buffer
                du = ep.tile([P, G], F32)
                nc.gpsimd.tensor_sub(du[:], uf3[:, :, 0], uf3[:, :, 1])
                w = ep.tile([P, G], F32)
                nc.gpsimd.tensor_mul(w[:], du[:], s[:])
                i1f = ep.tile([P, G], F32)
                nc.vector.tensor_scalar(
                    i1f[:], uf3[:, :, 0], -1.0, 255.0, OP.mult, OP.add
                )
                nc.gpsimd.tensor_add(ob[:, t * G:(t + 1) * G], i1f[:], w[:])

            nc.sync.dma_start(y_p, ob[:])

    with tile.TileContext(nc) as tc:
        with (
            tc.tile_pool(name="pp", bufs=1) as pp,
            tc.tile_pool(name="op", bufs=2) as op_,
        ):
            pat8 = pp.tile([P, D], U8)  # pat8[d] = 255 - d
            nc.gpsimd.iota(pat8[:], pattern=[[-1, D]], base=255,
                           channel_multiplier=0,
                           allow_small_or_imprecise_dtypes=True)
            ob = op_.tile([P, rpp], F32)
            if loop_iters == 1:
                body(tc, pat8, ob)
            else:
                with tc.For_i(0, loop_iters, 1):
                    body(tc, pat8, ob)

    nc.compile()
    return nc


_NC_CACHE = {}


def _get_nc(loop_iters: int = 1):
    if loop_iters not in _NC_CACHE:
        _NC_CACHE[loop_iters] = build(loop_iters)
    return _NC_CACHE[loop_iters]


def run(cost: np.ndarray, loop_iters: int = 1) -> np.ndarray:
    nc = _get_nc(loop_iters)
    flat = np.ascontiguousarray(cost.reshape(ROWS, D))
    in_maps = [
        {"cost": flat[c * ROWS_PER_CORE:(c + 1) * ROWS_PER_CORE]}
        for c in range(N_CORES)
    ]
    res = run_bass_kernel_spmd(nc, in_maps, core_ids=list(range(N_CORES)))
    out = np.concatenate(
        [res.results[c]["pred"] for c in range(N_CORES)]
    )
    return out.reshape(B, N).astype(np.float32, copy=False)


def kernel(cost: np.ndarray) -> np.ndarray:
    return run(cost, loop_iters=1)


# revision 11
# speedup vs baseline: 1.9070x; 1.9070x over previous
"""Trainium2 Bass kernel: disparity regression via top-2 over the last axis.

pred[b, n] = i1 + (i2 - i1) * sigmoid(v2 - v1)  with (v1,i1),(v2,i2) the top-2
of cost[b, n, :192] (ties -> lowest index, matching jax.lax.top_k).

Packed-key single-pass selection: overwrite the low byte of each fp32
element with (255 - d) via a strided uint8 copy (ACT engine), so
    key = (bits(x) & ~0xFF) | (255 - d)
Keys still compare like the values themselves under fp32 compare (the low
mantissa byte only perturbs the value by <= 256 ulp ~ 3e-5 relative), so a
single DVE max8 pass per row returns the top-2 values AND their indices:
i = 255 - (key & 0xFF), value ~= float(key | 0xFF). Quantization ties
resolve toward the lowest index for positive values (and toward the highest
for negative ones, where the reconstructed weights are ~0.5 anyway).

This needs no separate value pass (max_index) and no fp offset pass.
Measured per-core budget on HW: DVE ~173us (512 max8 at ~326ns + small
epilogue), ACT ~133us (byte-copy + sigmoid), Pool only issues SWDGE DMA.
The epilogue is batched once per iteration; per-tile epilogues stall the
in-order engine queues on cross-engine round trips. HW: 210us vs 316us
for the max8+max_index baseline.
"""
import numpy as np

import concourse.bacc as bacc
import concourse.tile as tile
import concourse.mybir as mybir
from concourse.bass_utils import run_bass_kernel_spmd

N_CORES = 8
B, N, D = 4, 131072, 192
ROWS = B * N                       # 524288
ROWS_PER_CORE = ROWS // N_CORES    # 65536
P = 128                            # SBUF partitions
G = 16                             # rows per partition per tile
TILE_ROWS = P * G                  # 2048
NQ = 2                             # DMA chunks per tile

F32 = mybir.dt.float32
I32 = mybir.dt.int32
U8 = mybir.dt.uint8
AF = mybir.ActivationFunctionType
OP = mybir.AluOpType


def build(loop_iters: int = 1, rows_per_core: int = ROWS_PER_CORE):
    nc = bacc.Bacc(
        "TRN2", target_bir_lowering=False, debug=False, num_devices=N_CORES
    )
    n_tiles = rows_per_core // TILE_ROWS
    rpp = rows_per_core // P  # rows per partition (contiguous run in DRAM)
    x = nc.dram_tensor("cost", [rows_per_core, D], F32, kind="ExternalInput").ap()
    y = nc.dram_tensor("pred", [rows_per_core], F32, kind="ExternalOutput").ap()

    # partition-major: partition p owns rows [p*rpp, (p+1)*rpp); tile t covers
    # rows p*rpp + t*G .. +G of every partition -> output is one contiguous
    # rpp-element run per partition (single efficient DMA per iteration).
    x_t = x.rearrange("(p t g) d -> t p (g d)", p=P, g=G)
    y_p = y.rearrange("(p r) -> p r", p=P)

    GD = G * D

    def body(tc, pat8, ob):
        pat_b = pat8[:].unsqueeze(1).to_broadcast([P, G, D])
        with (
            tc.tile_pool(name="xp", bufs=3) as xp,
            tc.tile_pool(name="sp", bufs=3) as sp,
            tc.tile_pool(name="kp", bufs=2) as kp,
            tc.tile_pool(name="ep", bufs=2) as ep,
        ):
            # top-2 packed keys of every row this iteration: [P, rpp, 2]
            kb = kp.tile([P, rpp * 2], F32)
            kb3 = kb[:].rearrange("p (r k) -> p r k", k=2)
            for t in range(n_tiles):
                xt = xp.tile([P, GD], F32)
                for q in range(NQ):
                    c0, c1 = q * GD // NQ, (q + 1) * GD // NQ
                    eng = nc.sync if q % 2 == 0 else nc.gpsimd
                    eng.dma_start(xt[:, c0:c1], x_t[t][:, c0:c1])

                # low byte := 255 - d  (little-endian byte 0 of each fp32)
                xb = xt[:].bitcast(U8).rearrange("p (g d four) -> p g d four",
                                                 d=D, four=4)
                nc.scalar.activation(xb[:, :, :, 0], pat_b, AF.Copy)

                # one max8 pass per row: top-2 packed keys in slots 0,1
                k8 = sp.tile([P, G * 8], F32)
                for g in range(G):
                    nc.vector.max(k8[:, g * 8:(g + 1) * 8],
                                  xt[:, g * D:(g + 1) * D])
                nc.vector.tensor_copy(
                    kb3[:, t * G:(t + 1) * G],
                    k8[:].rearrange("p (g k) -> p g k", k=8)[:, :, 0:2],
                )

            # batched epilogue over all rpp rows (few big instructions; no
            # per-tile cross-engine ping-pong on the in-order queues)
            ki = kb[:].bitcast(I32).rearrange("p (r k) -> p r k", k=2)

            # value part: vq = float(key | 0xFF);  dq = vq2 - vq1 (<= 0)
            mb = ep.tile([P, rpp * 2], I32)
            mb3 = mb[:].rearrange("p (r k) -> p r k", k=2)
            nc.vector.tensor_scalar(mb3, ki, 0xFF, None, OP.bitwise_or)
            mbf = mb[:].bitcast(F32).rearrange("p (r k) -> p r k", k=2)
            dq = ep.tile([P, rpp], F32)
            nc.vector.tensor_sub(dq[:], mbf[:, :, 1], mbf[:, :, 0])
            s = ep.tile([P, rpp], F32)
            nc.scalar.activation(s[:], dq[:], AF.Sigmoid)

            # index part: u = key & 0xFF;  i = 255 - u
            u = ep.tile([P, rpp * 2], I32)
            u3 = u[:].rearrange("p (r k) -> p r k", k=2)
            nc.vector.tensor_scalar(u3, ki, 0xFF, None, OP.bitwise_and)
            uf = ep.tile([P, rpp * 2], F32)
            uf3 = uf[:].rearrange("p (r k) -> p r k", k=2)
            nc.vector.tensor_copy(uf3, u3)

            # pred = (255-u1) + (u1-u2)*s
            du = ep.tile([P, rpp], F32)
            nc.vector.tensor_sub(du[:], uf3[:, :, 0], uf3[:, :, 1])
            w = ep.tile([P, rpp], F32)
            nc.vector.tensor_mul(w[:], du[:], s[:])
            i1f = ep.tile([P, rpp], F32)
            nc.vector.tensor_scalar(
                i1f[:], uf3[:, :, 0], -1.0, 255.0, OP.mult, OP.add
            )
            nc.vector.tensor_add(ob[:], i1f[:], w[:])

            nc.sync.dma_start(y_p, ob[:])

    with tile.TileContext(nc) as tc:
        with (
            tc.tile_pool(name="pp", bufs=1) as pp,
            tc.tile_pool(name="op", bufs=2) as op_,
        ):
            pat8 = pp.tile([P, D], U8)  # pat8[d] = 255 - d
            nc.gpsimd.iota(pat8[:], pattern=[[-1, D]], base=255,
                           channel_multiplier=0,
                           allow_small_or_imprecise_dtypes=True)
            ob = op_.tile([P, rpp], F32)
            if loop_iters == 1:
                body(tc, pat8, ob)
            else:
                with tc.For_i(0, loop_iters, 1):
                    body(tc, pat8, ob)

    nc.compile()
    return nc


_NC_CACHE = {}


def _get_nc(loop_iters: int = 1):
    if loop_iters not in _NC_CACHE:
        _NC_CACHE[loop_iters] = build(loop_iters)
    return _NC_CACHE[loop_iters]


def run(cost: np.ndarray, loop_iters: int = 1) -> np.ndarray:
    nc = _get_nc(loop_iters)
    flat = np.ascontiguousarray(cost.reshape(ROWS, D))
    in_maps = [
        {"cost": flat[c * ROWS_PER_CORE:(c + 1) * ROWS_PER_CORE]}
        for c in range(N_CORES)
    ]
    res = run_bass_kernel_spmd(nc, in_maps, core_ids=list(range(N_CORES)))
    out = np.concatenate(
        [res.results[c]["pred"] for c in range(N_CORES)]
    )
    return out.reshape(B, N).astype(np.float32, copy=False)


def kernel(cost: np.ndarray) -> np.ndarray:
    return run(cost, loop_iters=1)


# revision 12
# speedup vs baseline: 1.9910x; 1.0441x over previous
"""Trainium2 Bass kernel: disparity regression via top-2 over the last axis.

pred[b, n] = i1 + (i2 - i1) * sigmoid(v2 - v1)  with (v1,i1),(v2,i2) the top-2
of cost[b, n, :192] (ties -> lowest index, matching jax.lax.top_k).

Packed-key single-pass selection: overwrite the low byte of each fp32
element with (255 - d) via a strided uint8 copy (ACT engine), so
    key = (bits(x) & ~0xFF) | (255 - d)
Keys still compare like the values themselves under fp32 compare (the low
mantissa byte only perturbs the value by <= 256 ulp ~ 3e-5 relative), so a
single DVE max8 pass per row returns the top-2 values AND their indices:
i = 255 - (key & 0xFF), value ~= float(key | 0xFF). Quantization ties
resolve toward the lowest index for positive values (and toward the highest
for negative ones, where the reconstructed weights are ~0.5 anyway).

This needs no separate value pass (max_index) and no fp offset pass.
Measured per-core budget on HW: DVE ~173us (512 max8 at ~326ns + small
epilogue), ACT ~133us (byte-copy + sigmoid), Pool only issues SWDGE DMA.
The epilogue is batched once per iteration; per-tile epilogues stall the
in-order engine queues on cross-engine round trips. HW: 210us vs 316us
for the max8+max_index baseline.
"""
import numpy as np

import concourse.bacc as bacc
import concourse.tile as tile
import concourse.mybir as mybir
from concourse.bass_utils import run_bass_kernel_spmd

N_CORES = 8
B, N, D = 4, 131072, 192
ROWS = B * N                       # 524288
ROWS_PER_CORE = ROWS // N_CORES    # 65536
P = 128                            # SBUF partitions
G = 16                             # rows per partition per tile
TILE_ROWS = P * G                  # 2048
NQ = 2                             # DMA chunks per tile

F32 = mybir.dt.float32
I32 = mybir.dt.int32
U8 = mybir.dt.uint8
AF = mybir.ActivationFunctionType
OP = mybir.AluOpType


def build(loop_iters: int = 1, rows_per_core: int = ROWS_PER_CORE):
    nc = bacc.Bacc(
        "TRN2", target_bir_lowering=False, debug=False, num_devices=N_CORES
    )
    n_tiles = rows_per_core // TILE_ROWS
    rpp = rows_per_core // P  # rows per partition (contiguous run in DRAM)
    x = nc.dram_tensor("cost", [rows_per_core, D], F32, kind="ExternalInput").ap()
    y = nc.dram_tensor("pred", [rows_per_core], F32, kind="ExternalOutput").ap()

    # partition-major: partition p owns rows [p*rpp, (p+1)*rpp); tile t covers
    # rows p*rpp + t*G .. +G of every partition -> output is one contiguous
    # rpp-element run per partition (single efficient DMA per iteration).
    x_t = x.rearrange("(p t g) d -> t p (g d)", p=P, g=G)
    y_p = y.rearrange("(p r) -> p r", p=P)

    GD = G * D

    def body(tc, pat8, ob):
        pat_b = pat8[:].unsqueeze(1).to_broadcast([P, G, D])
        with (
            tc.tile_pool(name="xp", bufs=3) as xp,
            tc.tile_pool(name="sp", bufs=3) as sp,
            tc.tile_pool(name="kp", bufs=2) as kp,
            tc.tile_pool(name="ep", bufs=2) as ep,
        ):
            # top-2 packed keys of every row this iteration: [P, rpp, 2]
            kb = kp.tile([P, rpp * 2], F32)
            kb3 = kb[:].rearrange("p (r k) -> p r k", k=2)
            for t in range(n_tiles):
                xt = xp.tile([P, GD], F32)
                for q in range(NQ):
                    c0, c1 = q * GD // NQ, (q + 1) * GD // NQ
                    nc.sync.dma_start(xt[:, c0:c1], x_t[t][:, c0:c1])

                # low byte := 255 - d  (little-endian byte 0 of each fp32)
                xb = xt[:].bitcast(U8).rearrange("p (g d four) -> p g d four",
                                                 d=D, four=4)
                nc.scalar.activation(xb[:, :, :, 0], pat_b, AF.Copy)

                # one max8 pass per row: top-2 packed keys in slots 0,1
                k8 = sp.tile([P, G * 8], F32)
                for g in range(G):
                    nc.vector.max(k8[:, g * 8:(g + 1) * 8],
                                  xt[:, g * D:(g + 1) * D])
                nc.vector.tensor_copy(
                    kb3[:, t * G:(t + 1) * G],
                    k8[:].rearrange("p (g k) -> p g k", k=8)[:, :, 0:2],
                )

            # batched epilogue over all rpp rows (few big instructions; no
            # per-tile cross-engine ping-pong on the in-order queues)
            ki = kb[:].bitcast(I32).rearrange("p (r k) -> p r k", k=2)

            # value part: vq = float(key | 0xFF);  dq = vq2 - vq1 (<= 0)
            mb = ep.tile([P, rpp * 2], I32)
            mb3 = mb[:].rearrange("p (r k) -> p r k", k=2)
            nc.vector.tensor_scalar(mb3, ki, 0xFF, None, OP.bitwise_or)
            mbf = mb[:].bitcast(F32).rearrange("p (r k) -> p r k", k=2)
            dq = ep.tile([P, rpp], F32)
            nc.vector.tensor_sub(dq[:], mbf[:, :, 1], mbf[:, :, 0])
            s = ep.tile([P, rpp], F32)
            nc.scalar.activation(s[:], dq[:], AF.Sigmoid)

            # index part: u = key & 0xFF;  i = 255 - u
            u = ep.tile([P, rpp * 2], I32)
            u3 = u[:].rearrange("p (r k) -> p r k", k=2)
            nc.vector.tensor_scalar(u3, ki, 0xFF, None, OP.bitwise_and)
            uf = ep.tile([P, rpp * 2], F32)
            uf3 = uf[:].rearrange("p (r k) -> p r k", k=2)
            nc.vector.tensor_copy(uf3, u3)

            # pred = (255-u1) + (u1-u2)*s
            du = ep.tile([P, rpp], F32)
            nc.vector.tensor_sub(du[:], uf3[:, :, 0], uf3[:, :, 1])
            w = ep.tile([P, rpp], F32)
            nc.vector.tensor_mul(w[:], du[:], s[:])
            i1f = ep.tile([P, rpp], F32)
            nc.vector.tensor_scalar(
                i1f[:], uf3[:, :, 0], -1.0, 255.0, OP.mult, OP.add
            )
            nc.vector.tensor_add(ob[:], i1f[:], w[:])

            nc.sync.dma_start(y_p, ob[:])

    with tile.TileContext(nc) as tc:
        with (
            tc.tile_pool(name="pp", bufs=1) as pp,
            tc.tile_pool(name="op", bufs=2) as op_,
        ):
            pat8 = pp.tile([P, D], U8)  # pat8[d] = 255 - d
            nc.gpsimd.iota(pat8[:], pattern=[[-1, D]], base=255,
                           channel_multiplier=0,
                           allow_small_or_imprecise_dtypes=True)
            ob = op_.tile([P, rpp], F32)
            if loop_iters == 1:
                body(tc, pat8, ob)
            else:
                with tc.For_i(0, loop_iters, 1):
                    body(tc, pat8, ob)

    nc.compile()
    return nc


_NC_CACHE = {}


def _get_nc(loop_iters: int = 1):
    if loop_iters not in _NC_CACHE:
        _NC_CACHE[loop_iters] = build(loop_iters)
    return _NC_CACHE[loop_iters]


def run(cost: np.ndarray, loop_iters: int = 1) -> np.ndarray:
    nc = _get_nc(loop_iters)
    flat = np.ascontiguousarray(cost.reshape(ROWS, D))
    in_maps = [
        {"cost": flat[c * ROWS_PER_CORE:(c + 1) * ROWS_PER_CORE]}
        for c in range(N_CORES)
    ]
    res = run_bass_kernel_spmd(nc, in_maps, core_ids=list(range(N_CORES)))
    out = np.concatenate(
        [res.results[c]["pred"] for c in range(N_CORES)]
    )
    return out.reshape(B, N).astype(np.float32, copy=False)


def kernel(cost: np.ndarray) -> np.ndarray:
    return run(cost, loop_iters=1)


# revision 13
# speedup vs baseline: 2.3148x; 1.1626x over previous
"""Trainium2 Bass kernel: disparity regression via top-2 over the last axis.

pred[b, n] = i1 + (i2 - i1) * sigmoid(v2 - v1)  with (v1,i1),(v2,i2) the top-2
of cost[b, n, :192] (ties -> lowest index, matching jax.lax.top_k).

Packed-key single-pass selection: overwrite the low byte of each fp32
element with (255 - d) via a strided uint8 copy (ACT engine), so
    key = (bits(x) & ~0xFF) | (255 - d)
Keys still compare like the values themselves under fp32 compare (the low
mantissa byte only perturbs the value by <= 256 ulp ~ 3e-5 relative), so a
single DVE max8 pass per row returns the top-2 values AND their indices:
i = 255 - (key & 0xFF), value ~= float(key | 0xFF). Quantization ties
resolve toward the lowest index for positive values (and toward the highest
for negative ones, where the reconstructed weights are ~0.5 anyway).

This needs no separate value pass (max_index) and no fp offset pass.
Measured per-core budget on HW: DVE ~173us (512 max8 at ~326ns + small
epilogue), ACT ~133us (byte-copy + sigmoid), Pool only issues SWDGE DMA.
The epilogue is batched once per iteration; per-tile epilogues stall the
in-order engine queues on cross-engine round trips. HW: 210us vs 316us
for the max8+max_index baseline.
"""
import numpy as np

import concourse.bacc as bacc
import concourse.tile as tile
import concourse.mybir as mybir
from concourse.bass_utils import run_bass_kernel_spmd

N_CORES = 8
B, N, D = 4, 131072, 192
ROWS = B * N                       # 524288
ROWS_PER_CORE = ROWS // N_CORES    # 65536
P = 128                            # SBUF partitions
G = 16                             # rows per partition per tile
TILE_ROWS = P * G                  # 2048
NQ = 2                             # DMA chunks per tile

F32 = mybir.dt.float32
I32 = mybir.dt.int32
U8 = mybir.dt.uint8
AF = mybir.ActivationFunctionType
OP = mybir.AluOpType


def build(loop_iters: int = 1, rows_per_core: int = ROWS_PER_CORE):
    nc = bacc.Bacc(
        "TRN2", target_bir_lowering=False, debug=False, num_devices=N_CORES
    )
    n_tiles = rows_per_core // TILE_ROWS
    rpp = rows_per_core // P  # rows per partition (contiguous run in DRAM)
    x = nc.dram_tensor("cost", [rows_per_core, D], F32, kind="ExternalInput").ap()
    y = nc.dram_tensor("pred", [rows_per_core], F32, kind="ExternalOutput").ap()

    # partition-major: partition p owns rows [p*rpp, (p+1)*rpp); tile t covers
    # rows p*rpp + t*G .. +G of every partition -> output is one contiguous
    # rpp-element run per partition (single efficient DMA per iteration).
    x_t = x.rearrange("(p t g) d -> t p (g d)", p=P, g=G)
    y_p = y.rearrange("(p r) -> p r", p=P)

    GD = G * D

    def body(tc, pat8, ob):
        pat_b = pat8[:].unsqueeze(1).to_broadcast([P, G, D])
        with (
            tc.tile_pool(name="xp", bufs=4) as xp,
            tc.tile_pool(name="sp", bufs=3) as sp,
            tc.tile_pool(name="kp", bufs=2) as kp,
            tc.tile_pool(name="ep", bufs=2) as ep,
        ):
            # top-2 packed keys of every row this iteration: [P, rpp, 2]
            kb = kp.tile([P, rpp * 2], F32)
            kb3 = kb[:].rearrange("p (r k) -> p r k", k=2)
            for t in range(n_tiles):
                xt = xp.tile([P, GD], F32)
                for q in range(NQ):
                    c0, c1 = q * GD // NQ, (q + 1) * GD // NQ
                    nc.sync.dma_start(xt[:, c0:c1], x_t[t][:, c0:c1])

                # low byte := 255 - d  (little-endian byte 0 of each fp32)
                xb = xt[:].bitcast(U8).rearrange("p (g d four) -> p g d four",
                                                 d=D, four=4)
                nc.scalar.activation(xb[:, :, :, 0], pat_b, AF.Copy)

                # one max8 pass per row: top-2 packed keys in slots 0,1
                k8 = sp.tile([P, G * 8], F32)
                for g in range(G):
                    nc.vector.max(k8[:, g * 8:(g + 1) * 8],
                                  xt[:, g * D:(g + 1) * D])
                nc.vector.tensor_copy(
                    kb3[:, t * G:(t + 1) * G],
                    k8[:].rearrange("p (g k) -> p g k", k=8)[:, :, 0:2],
                )

            # batched epilogue over all rpp rows (few big instructions; no
            # per-tile cross-engine ping-pong on the in-order queues)
            ki = kb[:].bitcast(I32).rearrange("p (r k) -> p r k", k=2)

            # value part: vq = float(key | 0xFF);  dq = vq2 - vq1 (<= 0)
            mb = ep.tile([P, rpp * 2], I32)
            mb3 = mb[:].rearrange("p (r k) -> p r k", k=2)
            nc.vector.tensor_scalar(mb3, ki, 0xFF, None, OP.bitwise_or)
            mbf = mb[:].bitcast(F32).rearrange("p (r k) -> p r k", k=2)
            dq = ep.tile([P, rpp], F32)
            nc.vector.tensor_sub(dq[:], mbf[:, :, 1], mbf[:, :, 0])
            s = ep.tile([P, rpp], F32)
            nc.scalar.activation(s[:], dq[:], AF.Sigmoid)

            # index part: u = key & 0xFF;  i = 255 - u
            u = ep.tile([P, rpp * 2], I32)
            u3 = u[:].rearrange("p (r k) -> p r k", k=2)
            nc.vector.tensor_scalar(u3, ki, 0xFF, None, OP.bitwise_and)
            uf = ep.tile([P, rpp * 2], F32)
            uf3 = uf[:].rearrange("p (r k) -> p r k", k=2)
            nc.vector.tensor_copy(uf3, u3)

            # pred = (255-u1) + (u1-u2)*s
            du = ep.tile([P, rpp], F32)
            nc.vector.tensor_sub(du[:], uf3[:, :, 0], uf3[:, :, 1])
            w = ep.tile([P, rpp], F32)
            nc.vector.tensor_mul(w[:], du[:], s[:])
            i1f = ep.tile([P, rpp], F32)
            nc.vector.tensor_scalar(
                i1f[:], uf3[:, :, 0], -1.0, 255.0, OP.mult, OP.add
            )
            nc.vector.tensor_add(ob[:], i1f[:], w[:])

            nc.sync.dma_start(y_p, ob[:])

    with tile.TileContext(nc) as tc:
        with (
            tc.tile_pool(name="pp", bufs=1) as pp,
            tc.tile_pool(name="op", bufs=2) as op_,
        ):
            pat8 = pp.tile([P, D], U8)  # pat8[d] = 255 - d
            nc.gpsimd.iota(pat8[:], pattern=[[-1, D]], base=255,
                           channel_multiplier=0,
                           allow_small_or_imprecise_dtypes=True)
            ob = op_.tile([P, rpp], F32)
            if loop_iters == 1:
                body(tc, pat8, ob)
            else:
                with tc.For_i(0, loop_iters, 1):
                    body(tc, pat8, ob)

    nc.compile()
    return nc


_NC_CACHE = {}


def _get_nc(loop_iters: int = 1):
    if loop_iters not in _NC_CACHE:
        _NC_CACHE[loop_iters] = build(loop_iters)
    return _NC_CACHE[loop_iters]


def run(cost: np.ndarray, loop_iters: int = 1) -> np.ndarray:
    nc = _get_nc(loop_iters)
    flat = np.ascontiguousarray(cost.reshape(ROWS, D))
    in_maps = [
        {"cost": flat[c * ROWS_PER_CORE:(c + 1) * ROWS_PER_CORE]}
        for c in range(N_CORES)
    ]
    res = run_bass_kernel_spmd(nc, in_maps, core_ids=list(range(N_CORES)))
    out = np.concatenate(
        [res.results[c]["pred"] for c in range(N_CORES)]
    )
    return out.reshape(B, N).astype(np.float32, copy=False)


def kernel(cost: np.ndarray) -> np.ndarray:
    return run(cost, loop_iters=1)


# revision 14
# speedup vs baseline: 2.3288x; 1.0060x over previous
"""Trainium2 Bass kernel: disparity regression via top-2 over the last axis.

pred[b, n] = i1 + (i2 - i1) * sigmoid(v2 - v1)  with (v1,i1),(v2,i2) the top-2
of cost[b, n, :192] (ties -> lowest index, matching jax.lax.top_k).

Packed-key single-pass selection: overwrite the low byte of each fp32
element with (255 - d) via a strided uint8 copy (ACT engine), so
    key = (bits(x) & ~0xFF) | (255 - d)
Keys still compare like the values themselves under fp32 compare (the low
mantissa byte only perturbs the value by <= 256 ulp ~ 3e-5 relative), so a
single DVE max8 pass per row returns the top-2 values AND their indices:
i = 255 - (key & 0xFF), value ~= float(key | 0xFF). Quantization ties
resolve toward the lowest index for positive values (and toward the highest
for negative ones, where the reconstructed weights are ~0.5 anyway).

This needs no separate value pass (max_index) and no fp offset pass.
Measured per-core budget on HW: DVE ~173us (512 max8 at ~326ns + small
epilogue), ACT ~133us (byte-copy + sigmoid), Pool only issues SWDGE DMA.
The epilogue is batched once per iteration; per-tile epilogues stall the
in-order engine queues on cross-engine round trips. HW: 210us vs 316us
for the max8+max_index baseline.
"""
import numpy as np

import concourse.bacc as bacc
import concourse.tile as tile
import concourse.mybir as mybir
from concourse.bass_utils import run_bass_kernel_spmd

N_CORES = 8
B, N, D = 4, 131072, 192
ROWS = B * N                       # 524288
ROWS_PER_CORE = ROWS // N_CORES    # 65536
P = 128                            # SBUF partitions
G = 16                             # rows per partition per tile
TILE_ROWS = P * G                  # 2048
NQ = 2                             # DMA chunks per tile

F32 = mybir.dt.float32
I32 = mybir.dt.int32
U8 = mybir.dt.uint8
AF = mybir.ActivationFunctionType
OP = mybir.AluOpType


def build(loop_iters: int = 1, rows_per_core: int = ROWS_PER_CORE):
    nc = bacc.Bacc(
        "TRN2", target_bir_lowering=False, debug=False, num_devices=N_CORES
    )
    n_tiles = rows_per_core // TILE_ROWS
    rpp = rows_per_core // P  # rows per partition (contiguous run in DRAM)
    x = nc.dram_tensor("cost", [rows_per_core, D], F32, kind="ExternalInput").ap()
    y = nc.dram_tensor("pred", [rows_per_core], F32, kind="ExternalOutput").ap()

    # partition-major: partition p owns rows [p*rpp, (p+1)*rpp); tile t covers
    # rows p*rpp + t*G .. +G of every partition -> output is one contiguous
    # rpp-element run per partition (single efficient DMA per iteration).
    x_t = x.rearrange("(p t g) d -> t p (g d)", p=P, g=G)
    y_p = y.rearrange("(p r) -> p r", p=P)

    GD = G * D

    def body(tc, pat8, ob):
        pat_b = pat8[:].unsqueeze(1).to_broadcast([P, G, D])
        with (
            tc.tile_pool(name="xp", bufs=6) as xp,
            tc.tile_pool(name="sp", bufs=3) as sp,
            tc.tile_pool(name="kp", bufs=2) as kp,
            tc.tile_pool(name="ep", bufs=2) as ep,
        ):
            # top-2 packed keys of every row this iteration: [P, rpp, 2]
            kb = kp.tile([P, rpp * 2], F32)
            kb3 = kb[:].rearrange("p (r k) -> p r k", k=2)
            for t in range(n_tiles):
                xt = xp.tile([P, GD], F32)
                for q in range(NQ):
                    c0, c1 = q * GD // NQ, (q + 1) * GD // NQ
                    nc.sync.dma_start(xt[:, c0:c1], x_t[t][:, c0:c1])

                # low byte := 255 - d  (little-endian byte 0 of each fp32)
                xb = xt[:].bitcast(U8).rearrange("p (g d four) -> p g d four",
                                                 d=D, four=4)
                nc.scalar.activation(xb[:, :, :, 0], pat_b, AF.Copy)

                # one max8 pass per row: top-2 packed keys in slots 0,1
                k8 = sp.tile([P, G * 8], F32)
                for g in range(G):
                    nc.vector.max(k8[:, g * 8:(g + 1) * 8],
                                  xt[:, g * D:(g + 1) * D])
                nc.vector.tensor_copy(
                    kb3[:, t * G:(t + 1) * G],
                    k8[:].rearrange("p (g k) -> p g k", k=8)[:, :, 0:2],
                )

            # batched epilogue over all rpp rows (few big instructions; no
            # per-tile cross-engine ping-pong on the in-order queues)
            ki = kb[:].bitcast(I32).rearrange("p (r k) -> p r k", k=2)

            # value part: vq = float(key | 0xFF);  dq = vq2 - vq1 (<= 0)
            mb = ep.tile([P, rpp * 2], I32)
            mb3 = mb[:].rearrange("p (r k) -> p r k", k=2)
            nc.vector.tensor_scalar(mb3, ki, 0xFF, None, OP.bitwise_or)
            mbf = mb[:].bitcast(F32).rearrange("p (r k) -> p r k", k=2)
            dq = ep.tile([P, rpp], F32)
            nc.vector.tensor_sub(dq[:], mbf[:, :, 1], mbf[:, :, 0])
            s = ep.tile([P, rpp], F32)
            nc.scalar.activation(s[:], dq[:], AF.Sigmoid)

            # index part: u = key & 0xFF;  i = 255 - u
            u = ep.tile([P, rpp * 2], I32)
            u3 = u[:].rearrange("p (r k) -> p r k", k=2)
            nc.vector.tensor_scalar(u3, ki, 0xFF, None, OP.bitwise_and)
            uf = ep.tile([P, rpp * 2], F32)
            uf3 = uf[:].rearrange("p (r k) -> p r k", k=2)
            nc.vector.tensor_copy(uf3, u3)

            # pred = (255-u1) + (u1-u2)*s
            du = ep.tile([P, rpp], F32)
            nc.vector.tensor_sub(du[:], uf3[:, :, 0], uf3[:, :, 1])
            w = ep.tile([P, rpp], F32)
            nc.vector.tensor_mul(w[:], du[:], s[:])
            i1f = ep.tile([P, rpp], F32)
            nc.vector.tensor_scalar(
                i1f[:], uf3[:, :, 0], -1.0, 255.0, OP.mult, OP.add
            )
            nc.vector.tensor_add(ob[:], i1f[:], w[:])

            nc.sync.dma_start(y_p, ob[:])

    with tile.TileContext(nc) as tc:
        with (
            tc.tile_pool(name="pp", bufs=1) as pp,
            tc.tile_pool(name="op", bufs=2) as op_,
        ):
            pat8 = pp.tile([P, D], U8)  # pat8[d] = 255 - d
            nc.gpsimd.iota(pat8[:], pattern=[[-1, D]], base=255,
                           channel_multiplier=0,
                           allow_small_or_imprecise_dtypes=True)
            ob = op_.tile([P, rpp], F32)
            if loop_iters == 1:
                body(tc, pat8, ob)
            else:
                with tc.For_i(0, loop_iters, 1):
                    body(tc, pat8, ob)

    nc.compile()
    return nc


_NC_CACHE = {}


def _get_nc(loop_iters: int = 1):
    if loop_iters not in _NC_CACHE:
        _NC_CACHE[loop_iters] = build(loop_iters)
    return _NC_CACHE[loop_iters]


def run(cost: np.ndarray, loop_iters: int = 1) -> np.ndarray:
    nc = _get_nc(loop_iters)
    flat = np.ascontiguousarray(cost.reshape(ROWS, D))
    in_maps = [
        {"cost": flat[c * ROWS_PER_CORE:(c + 1) * ROWS_PER_CORE]}
        for c in range(N_CORES)
    ]
    res = run_bass_kernel_spmd(nc, in_maps, core_ids=list(range(N_CORES)))
    out = np.concatenate(
        [res.results[c]["pred"] for c in range(N_CORES)]
    )
    return out.reshape(B, N).astype(np.float32, copy=False)


def kernel(cost: np.ndarray) -> np.ndarray:
    return run(cost, loop_iters=1)
